# revision 25
# baseline (speedup 1.0000x reference)
"""8-core Trainium2 kernel for a 6-layer transformer LM (MuP attention).

Sharding: token-parallel. Core k owns 256 consecutive tokens (batch k//4,
chunks 2c,2c+1 with c=k%4). Per-layer AllGather of K^T/V inside each
batch-group of 4 cores; vocab-sharded head after an 8-way AllGather of the
final hidden state. Causality is SPMD-uniform: per-core bias data masks
invalid key slots inside exp(), diagonal blocks use local K/V with a static
triangular mask, and a ones-column on V produces softmax denominators.
"""

import os
import numpy as np
import ml_dtypes

import concourse.bass as bass
import concourse.tile as tile
import concourse.mybir as mybir
from concourse import bacc
from concourse.bass_utils import run_bass_kernel_spmd

BF16 = mybir.dt.bfloat16
F32 = mybir.dt.float32

V, D, L, H, F, S = 32000, 1024, 6, 16, 4096, 1024
B, T = 2, 1024
DH = D // H          # 64
P = 128
N_CORES = 8
TOK = 256            # tokens per core
QC = 2               # q-chunks of 128 per core
KO = D // P          # 8
FO = F // 128        # 32
VSLICE = V // N_CORES    # 4000
VS = 8               # vocab sub-slices per core
VSW = VSLICE // VS   # 500
NEG = -50.0
EPS = 1e-5

N_LAYERS = int(os.environ.get("KL_LAYERS", str(L)))
WITH_HEAD = os.environ.get("KL_HEAD", "1") == "1"


def build_graph():
    nc = bacc.Bacc(None, target_bir_lowering=False)

    x0_t = nc.dram_tensor("x0", [TOK, D], F32, kind="ExternalInput")
    attb_t = nc.dram_tensor("attb", [8], F32, kind="ExternalInput")
    wlin_t = nc.dram_tensor("wlin", [L, 4, D, D], BF16, kind="ExternalInput")
    w1t_t = nc.dram_tensor("w1t", [L, D, F], BF16, kind="ExternalInput")
    w2t_t = nc.dram_tensor("w2t", [L, F, D], BF16, kind="ExternalInput")
    b1_t = nc.dram_tensor("b1", [L, F], BF16, kind="ExternalInput")
    b2_t = nc.dram_tensor("b2", [L, D], BF16, kind="ExternalInput")
    lnw_t = nc.dram_tensor("lnw", [L, 4, D], BF16, kind="ExternalInput")
    lnf_t = nc.dram_tensor("lnf", [2, D], BF16, kind="ExternalInput")
    hwt_t = nc.dram_tensor("hwt", [D, VSLICE], BF16, kind="ExternalInput")
    if WITH_HEAD:
        out_t = nc.dram_tensor("logits", [B * T, VSLICE], F32, kind="ExternalOutput")
    else:
        out_t = nc.dram_tensor("xout", [TOK, D], F32, kind="ExternalOutput")
    DEBUG = os.environ.get("KL_DEBUG", "0") == "1"
    if DEBUG:
        dqt_t = nc.dram_tensor("dqt", [D, TOK], BF16, kind="ExternalOutput")
        dkt_t = nc.dram_tensor("dkt", [D, 4 * TOK], BF16, kind="ExternalOutput")
        dvf_t = nc.dram_tensor("dvf", [8 * P, H * (DH + 2)], BF16, kind="ExternalOutput")
        de_t = nc.dram_tensor("de", [8 * P, TOK], BF16, kind="ExternalOutput")
        del_t = nc.dram_tensor("del_", [3 * P, P], BF16, kind="ExternalOutput")
        dden_t = nc.dram_tensor("dden", [P, QC], F32, kind="ExternalOutput")
        du_t = nc.dram_tensor("du", [P, QC, DH + 2], F32, kind="ExternalOutput")
        dy_t = nc.dram_tensor("dy", [TOK, D], BF16, kind="ExternalOutput")

    groups4 = [[0, 1, 2, 3], [4, 5, 6, 7]]
    groups8 = [list(range(8))]

    from contextlib import ExitStack
    with tile.TileContext(nc) as tc, ExitStack() as ctx:
        const = ctx.enter_context(tc.tile_pool(name="const", bufs=1))
        dram = ctx.enter_context(tc.tile_pool(name="dram", bufs=1, space="DRAM"))
        wt = ctx.enter_context(tc.tile_pool(name="wt", bufs=3))
        act = ctx.enter_context(tc.tile_pool(name="act", bufs=1))
        kv = ctx.enter_context(tc.tile_pool(name="kv", bufs=1))
        ep = ctx.enter_context(tc.tile_pool(name="ep", bufs=2))
        small = ctx.enter_context(tc.tile_pool(name="small", bufs=4))
        outp = ctx.enter_context(tc.tile_pool(name="outp", bufs=2))
        xp = ctx.enter_context(tc.tile_pool(name="xp", bufs=1))
        ps_big = ctx.enter_context(tc.tile_pool(name="ps_big", bufs=3, space="PSUM"))
        ps_sc = ctx.enter_context(tc.tile_pool(name="ps_sc", bufs=2, space="PSUM"))
        ps_u = ctx.enter_context(tc.tile_pool(name="ps_u", bufs=2, space="PSUM"))
        ps_tr = ctx.enter_context(tc.tile_pool(name="ps_tr", bufs=1, space="PSUM"))

        # ---- constants ----
        ident = const.tile([P, P], BF16)
        nc.gpsimd.memset(ident, 0.0)
        nc.gpsimd.affine_select(
            out=ident, in_=ident, compare_op=mybir.AluOpType.not_equal,
            fill=1.0, base=0, pattern=[[-1, P]], channel_multiplier=1)
        # trimask[k, q] = 0 if q >= k else NEG   (valid = k <= q)
        trimask = const.tile([P, P], F32)
        nc.gpsimd.memset(trimask, 0.0)
        nc.gpsimd.affine_select(
            out=trimask, in_=trimask, compare_op=mybir.AluOpType.is_ge,
            fill=NEG * DH, base=0, pattern=[[1, P]], channel_multiplier=-1)
        ones1 = const.tile([1, P], BF16)
        nc.vector.memset(ones1, 1.0)
        eps_sb = const.tile([P, 1], F32)
        nc.vector.memset(eps_sb, EPS)
        # attention slot bias, broadcast to all partitions: [128, 8]
        attb_sb = const.tile([P, 8], F32)
        nc.gpsimd.dma_start(attb_sb, bass.AP(tensor=attb_t, offset=0, ap=[[0, P], [1, 8]]))

        # ---- residual stream ----
        x_sb = xp.tile([P, QC, D], F32)
        nc.sync.dma_start(x_sb, x0_t[:].rearrange("(qc p) d -> p qc d", p=P))

        def layer_norm(w_b, b_b, out_bf):
            """x_sb -> out_bf (bf16 [P, QC, D]); w_b/b_b broadcast [P, D] bf16."""
            for qc in range(QC):
                stats = small.tile([P, 2, 6], F32, tag="stats")
                nc.vector.bn_stats(stats[:, 0], x_sb[:, qc, 0:512])
                nc.vector.bn_stats(stats[:, 1], x_sb[:, qc, 512:1024])
                mv = small.tile([P, 2], F32, tag="mv")
                nc.vector.bn_aggr(mv, stats)
                sd = small.tile([P, 1], F32, tag="sd")
                nc.scalar.activation(sd, mv[:, 1:2], mybir.ActivationFunctionType.Sqrt,
                                     bias=eps_sb, scale=1.0)
                rs = small.tile([P, 1], F32, tag="rs")
                nc.vector.reciprocal(rs, sd)
                t = act.tile([P, D], F32, tag="lnt")
                nc.vector.tensor_scalar(
                    t, x_sb[:, qc], scalar1=mv[:, 0:1], scalar2=rs,
                    op0=mybir.AluOpType.subtract, op1=mybir.AluOpType.mult)
                nc.vector.tensor_mul(t, t, w_b)
                nc.vector.tensor_add(out_bf[:, qc], t, b_b)

        def transpose_blocks(src, dst, nko):
            """src bf16 [P, QC, nko*128]; dst bf16 [P, nko, 256] (transposed)."""
            for qc in range(QC):
                for ko in range(nko):
                    pt = ps_tr.tile([P, P], BF16, tag="tr")
                    nc.tensor.transpose(pt, src[:, qc, ko * P:(ko + 1) * P], ident)
                    nc.any.tensor_copy(dst[:, ko, qc * P:(qc + 1) * P], pt)

        def load_w(dram_ap):
            w_sb = wt.tile([P, KO, 1024], BF16, tag="wt")
            nc.sync.dma_start(w_sb, dram_ap)
            return w_sb

        for l in range(N_LAYERS):
            # ---- LN weights (broadcast across partitions) ----
            lnw_sb = act.tile([P, 4, D], BF16, tag="lnw")
            nc.gpsimd.dma_start(
                lnw_sb, bass.AP(tensor=lnw_t, offset=l * 4 * D, ap=[[0, P], [D, 4], [1, D]]))

            # ---- LN1 -> h ----
            h_bf = act.tile([P, QC, D], BF16, tag="h")
            layer_norm(lnw_sb[:, 0], lnw_sb[:, 1], h_bf)
            hT = act.tile([P, KO, TOK], BF16, tag="hT", bufs=2)
            transpose_blocks(h_bf, hT, KO)

            # ---- K^T, V (local), then AllGather ----
            wk_sb = load_w(wlin_t[l, 1].rearrange("(ko p) n -> p ko n", p=P))
            KT_own = act.tile([P, KO, TOK], BF16, tag="KT_own")
            for mo in range(KO):
                pk = ps_big.tile([P, TOK], F32, tag="big")
                for ki in range(KO):
                    nc.tensor.matmul(pk, wk_sb[:, ki, mo * P:(mo + 1) * P], hT[:, ki],
                                     start=(ki == 0), stop=(ki == KO - 1))
                nc.any.tensor_copy(KT_own[:, mo], pk)

            wv_sb = load_w(wlin_t[l, 2].rearrange("(ko p) n -> p ko n", p=P))
            V_own = act.tile([P, QC, H, DH + 2], BF16, tag="V_own")
            nc.vector.memset(V_own[:, :, :, DH:DH + 2], 1.0)
            for qc in range(QC):
                for no in range(2):
                    pv = ps_big.tile([P, 512], F32, tag="big")
                    for ki in range(KO):
                        nc.tensor.matmul(pv, hT[:, ki, qc * P:(qc + 1) * P],
                                         wv_sb[:, ki, no * 512:(no + 1) * 512],
                                         start=(ki == 0), stop=(ki == KO - 1))
                    nc.any.tensor_copy(V_own[:, qc, 8 * no:8 * (no + 1), 0:DH],
                                       pv[:].rearrange("p (h d) -> p h d", d=DH))

            kt_in = dram.tile([D, TOK], BF16, tag="kt_in")
            kt_out = dram.tile([4, D, TOK], BF16, tag="kt_out")
            v_in = dram.tile([TOK, D], BF16, tag="v_in")
            v_out = dram.tile([4, TOK, D], BF16, tag="v_out")
            nc.sync.dma_start(kt_in[:].rearrange("(ko p) t -> p ko t", p=P), KT_own)
            for qc in range(QC):
                nc.sync.dma_start(
                    v_in[:].rearrange("(qc p) (h d) -> p qc h d", p=P, h=H)[:, qc],
                    V_own[:, qc, :, 0:DH])
            nc.gpsimd.collective_compute(
                "AllGather", mybir.AluOpType.bypass, replica_groups=groups4,
                ins=[kt_in.opt()], outs=[kt_out.opt()])
            nc.gpsimd.collective_compute(
                "AllGather", mybir.AluOpType.bypass, replica_groups=groups4,
                ins=[v_in.opt()], outs=[v_out.opt()])

            # ---- Q^T (overlaps the collectives) ----
            wq_sb = load_w(wlin_t[l, 0].rearrange("(ko p) n -> p ko n", p=P))
            QT = act.tile([P, KO, TOK], BF16, tag="QT")
            for mo in range(KO):
                pq = ps_big.tile([P, TOK], F32, tag="big")
                for ki in range(KO):
                    nc.tensor.matmul(pq, wq_sb[:, ki, mo * P:(mo + 1) * P], hT[:, ki],
                                     start=(ki == 0), stop=(ki == KO - 1))
                nc.any.tensor_copy(QT[:, mo], pq)

            # ---- gathered K^T / V ----
            KT_full = kv.tile([P, KO, 4 * TOK], BF16, tag="KT_full", bufs=1)
            for r in range(4):
                nc.sync.dma_start(
                    KT_full[:, :, r * TOK:(r + 1) * TOK],
                    kt_out[r].rearrange("(ko p) t -> p ko t", p=P))
            V_full = kv.tile([P, 8, H, DH + 2], BF16, tag="V_full")
            nc.vector.memset(V_full[:, :, :, DH:DH + 2], 1.0)
            for r in range(4):
                for qc in range(QC):
                    nc.sync.dma_start(
                        V_full[:, 2 * r + qc, :, 0:DH],
                        v_out[r].rearrange("(qc p) (h d) -> p qc h d", p=P, h=H)[:, qc])

            # ---- attention ----
            y_bf = act.tile([P, QC, H, DH], BF16, tag="y")
            exp_f = mybir.ActivationFunctionType.Exp
            for h in range(H):
                prow = slice((h % 2) * DH, (h % 2) * DH + DH)
                ko_h = h // 2
                E = ep.tile([P, 8, TOK], BF16, tag="E")
                for g in range(8):
                    sc = ps_sc.tile([P, TOK], F32, tag="sc")
                    nc.tensor.matmul(sc, KT_full[prow, ko_h, g * P:(g + 1) * P],
                                     QT[prow, ko_h], start=True, stop=True)
                    nc.scalar.activation(E[:, g], sc, exp_f,
                                         bias=attb_sb[:, g:g + 1], scale=1.0 / DH)
                El = ep.tile([P, 3, P], BF16, tag="El")
                for j, (qc, kc, tri) in enumerate([(0, 0, True), (1, 0, False), (1, 1, True)]):
                    scl = ps_sc.tile([P, TOK], F32, tag="sc", name="scl")[:, 0:P]
                    nc.tensor.matmul(scl, KT_own[prow, ko_h, kc * P:(kc + 1) * P],
                                     QT[prow, ko_h, qc * P:(qc + 1) * P],
                                     start=True, stop=True)
                    if tri:
                        nc.vector.tensor_add(scl, scl, trimask)
                    nc.scalar.activation(El[:, j], scl, exp_f, bias=0.0, scale=1.0 / DH)
                if DEBUG and l == 0 and h == 0:
                    nc.sync.dma_start(de_t[:].rearrange("(g p) t -> p g t", p=P), E)
                    nc.sync.dma_start(del_t[:].rearrange("(j p) t -> p j t", p=P), El)
                for qc in range(QC):
                    U = ps_u.tile([P, DH + 2], F32, tag="u")
                    for g in range(8):
                        nc.tensor.matmul(U, E[:, g, qc * P:(qc + 1) * P],
                                         V_full[:, g, h], start=(g == 0), stop=False)
                    locs = [(0, 0)] if qc == 0 else [(1, 0), (2, 1)]
                    for i, (j, kc) in enumerate(locs):
                        nc.tensor.matmul(U, El[:, j], V_own[:, kc, h],
                                         start=False, stop=(i == len(locs) - 1))
                    rec = small.tile([P, 1], F32, tag="rec")
                    nc.vector.reciprocal(rec, U[:, DH:DH + 1])
                    if DEBUG and l == 0 and h == 0:
                        dden_sb = small.tile([P, 1], F32, tag="dden")
                        nc.vector.tensor_copy(dden_sb, U[:, DH:DH + 1])
                        nc.sync.dma_start(dden_t[:, qc:qc + 1], dden_sb)
                        du_sb = small.tile([P, DH + 2], F32, tag="du")
                        nc.vector.tensor_copy(du_sb, U)
                        nc.sync.dma_start(du_t[:, qc], du_sb)
                    nc.vector.tensor_scalar_mul(y_bf[:, qc, h], U[:, 0:DH], rec)

            if DEBUG and l == 0:
                nc.sync.dma_start(dqt_t[:].rearrange("(ko p) t -> p ko t", p=P), QT)
                nc.sync.dma_start(dkt_t[:].rearrange("(ko p) t -> p ko t", p=P), KT_full)
                nc.sync.dma_start(dvf_t[:].rearrange("(g p) x -> p g x", p=P),
                                  V_full[:].rearrange("p g h d -> p g (h d)"))
                nc.sync.dma_start(dy_t[:].rearrange("(qc p) d -> p qc d", p=P),
                                  y_bf[:].rearrange("p qc h d -> p qc (h d)"))

            # ---- proj + residual ----
            yT = act.tile([P, KO, TOK], BF16, tag="hT", bufs=2)
            transpose_blocks(y_bf[:].rearrange("p qc h d -> p qc (h d)"), yT, KO)
            wp_sb = load_w(wlin_t[l, 3].rearrange("(ko p) n -> p ko n", p=P))
            for qc in range(QC):
                for no in range(2):
                    pp = ps_big.tile([P, 512], F32, tag="big")
                    for ki in range(KO):
                        nc.tensor.matmul(pp, yT[:, ki, qc * P:(qc + 1) * P],
                                         wp_sb[:, ki, no * 512:(no + 1) * 512],
                                         start=(ki == 0), stop=(ki == KO - 1))
                    nc.vector.tensor_add(x_sb[:, qc, no * 512:(no + 1) * 512],
                                         x_sb[:, qc, no * 512:(no + 1) * 512], pp)

            # ---- LN2 -> FFN ----
            h2_bf = act.tile([P, QC, D], BF16, tag="h")
            layer_norm(lnw_sb[:, 2], lnw_sb[:, 3], h2_bf)
            h2T = act.tile([P, KO, TOK], BF16, tag="hT", bufs=2)
            transpose_blocks(h2_bf, h2T, KO)

            b1_sb = small.tile([1, F], BF16, tag="b1", bufs=1)
            nc.sync.dma_start(b1_sb, b1_t[l][None, :])
            g_bf = act.tile([P, QC, F], BF16, tag="g")
            for q4 in range(4):
                w1_sb = wt.tile([P, KO, 1024], BF16, tag="wt")
                nc.sync.dma_start(
                    w1_sb,
                    w1t_t[l].rearrange("(ko p) n -> p ko n", p=P)[:, :, q4 * 1024:(q4 + 1) * 1024])
                for qc in range(QC):
                    for f2 in range(2):
                        pf = ps_big.tile([P, 512], F32, tag="big")
                        for ki in range(KO):
                            nc.tensor.matmul(pf, h2T[:, ki, qc * P:(qc + 1) * P],
                                             w1_sb[:, ki, f2 * 512:(f2 + 1) * 512],
                                             start=(ki == 0), stop=False)
                        fa = q4 * 1024 + f2 * 512
                        nc.tensor.matmul(pf, ones1, b1_sb[:, fa:fa + 512],
                                         start=False, stop=True)
                        nc.scalar.activation(g_bf[:, qc, fa:fa + 512], pf,
                                             mybir.ActivationFunctionType.Gelu)

            gT = act.tile([P, FO, TOK], BF16, tag="gT")
            for qc in range(QC):
                for j in range(FO):
                    pt = ps_tr.tile([P, P], BF16, tag="tr")
                    nc.tensor.transpose(pt, g_bf[:, qc, j * P:(j + 1) * P], ident)
                    nc.any.tensor_copy(gT[:, j, qc * P:(qc + 1) * P], pt)

            b2_sb = small.tile([1, D], BF16, tag="b2", bufs=1)
            nc.sync.dma_start(b2_sb, b2_t[l][None, :])
            for ou in range(4):
                w2_sb = wt.tile([P, FO, TOK], BF16, tag="wt")
                nc.sync.dma_start(
                    w2_sb,
                    w2t_t[l].rearrange("(ko p) n -> p ko n", p=P)[:, :, ou * TOK:(ou + 1) * TOK])
                for qc in range(QC):
                    pf2 = ps_big.tile([P, 512], F32, tag="big", name="pf2")[:, 0:TOK]
                    for kj in range(FO):
                        nc.tensor.matmul(pf2, gT[:, kj, qc * P:(qc + 1) * P],
                                         w2_sb[:, kj], start=(kj == 0), stop=False)
                    nc.tensor.matmul(pf2, ones1, b2_sb[:, ou * TOK:(ou + 1) * TOK],
                                     start=False, stop=True)
                    nc.vector.tensor_add(x_sb[:, qc, ou * TOK:(ou + 1) * TOK],
                                         x_sb[:, qc, ou * TOK:(ou + 1) * TOK], pf2)

        if not WITH_HEAD:
            nc.sync.dma_start(out_t[:].rearrange("(qc p) d -> p qc d", p=P), x_sb)
        else:
            # ---- final LN, transpose, 8-way AllGather ----
            lnf_sb = act.tile([P, 4, D], BF16, tag="lnw")
            nc.gpsimd.dma_start(
                lnf_sb[:, 0:2], bass.AP(tensor=lnf_t, offset=0, ap=[[0, P], [D, 2], [1, D]]))
            xf_bf = act.tile([P, QC, D], BF16, tag="h")
            layer_norm(lnf_sb[:, 0], lnf_sb[:, 1], xf_bf)
            xfT = act.tile([P, KO, TOK], BF16, tag="hT", bufs=2)
            transpose_blocks(xf_bf, xfT, KO)
            xf_in = dram.tile([D, TOK], BF16, tag="xf_in")
            xf_out = dram.tile([8, D, TOK], BF16, tag="xf_out")
            nc.sync.dma_start(xf_in[:].rearrange("(ko p) t -> p ko t", p=P), xfT)
            nc.gpsimd.collective_compute(
                "AllGather", mybir.AluOpType.bypass, replica_groups=groups8,
                ins=[xf_in.opt()], outs=[xf_out.opt()])

            # ---- head: [2048, 1024] @ [1024, 4000], token-halved ----
            for th in range(2):
                XF = kv.tile([P, KO, 4 * TOK], BF16, tag="KT_full", bufs=1)
                for r in range(4):
                    nc.sync.dma_start(
                        XF[:, :, r * TOK:(r + 1) * TOK],
                        xf_out[4 * th + r].rearrange("(ko p) t -> p ko t", p=P))
                for vs in range(VS):
                    hw_sb = wt.tile([P, KO, VSW], BF16, tag="hw", bufs=2)
                    nc.sync.dma_start(
                        hw_sb,
                        hwt_t[:].rearrange("(ko p) v -> p ko v", p=P)[:, :, vs * VSW:(vs + 1) * VSW])
                    for to in range(8):
                        ph = ps_big.tile([P, 512], F32, tag="big", name="ph")[:, 0:VSW]
                        for ki in range(KO):
                            nc.tensor.matmul(ph, XF[:, ki, to * P:(to + 1) * P],
                                             hw_sb[:, ki], start=(ki == 0), stop=(ki == KO - 1))
                        ot = outp.tile([P, VSW], F32, tag="ot")
                        nc.any.tensor_copy(ot, ph)
                        nc.sync.dma_start(
                            out_t[(8 * th + to) * P:(8 * th + to + 1) * P,
                                  vs * VSW:(vs + 1) * VSW], ot)

    nc.compile()
    return nc


_CACHED_NC = None
_LAST_RES = None


def prepare_in_maps(idx, tok_emb, pos_emb, ln1_w, ln1_b, wq, wk, wv, proj,
                    ln2_w, ln2_b, fc1_w, fc1_b, fc2_w, fc2_b, lnf_w, lnf_b, head_w):
    bf = ml_dtypes.bfloat16
    x0 = tok_emb[np.asarray(idx)] + pos_emb[:T][None, :, :]   # [B, T, D] f32
    x0 = np.ascontiguousarray(x0, dtype=np.float32)

    wlin = np.stack([wq, wk, wv, proj], axis=1).transpose(0, 1, 3, 2)  # [L,4,D,D] (d_in, d_out)
    wlin = np.ascontiguousarray(wlin).astype(bf)
    w1t = np.ascontiguousarray(fc1_w.transpose(0, 2, 1)).astype(bf)   # [L, D, F]
    w2t = np.ascontiguousarray(fc2_w.transpose(0, 2, 1)).astype(bf)   # [L, F, D]
    b1 = fc1_b.astype(bf)
    b2 = fc2_b.astype(bf)
    lnw = np.ascontiguousarray(np.stack([ln1_w, ln1_b, ln2_w, ln2_b], axis=1)).astype(bf)
    lnf = np.ascontiguousarray(np.stack([lnf_w, lnf_b])).astype(bf)
    hwt_full = np.ascontiguousarray(head_w.T).astype(bf)               # [D, V]

    in_maps = []
    for k in range(N_CORES):
        g, c = k // 4, k % 4
        attb = np.where(np.arange(8) < 2 * c, 0.0, NEG).astype(np.float32)
        in_maps.append({
            "x0": np.ascontiguousarray(x0[g, TOK * c: TOK * (c + 1)]),
            "attb": attb,
            "wlin": wlin, "w1t": w1t, "w2t": w2t, "b1": b1, "b2": b2,
            "lnw": lnw, "lnf": lnf,
            "hwt": np.ascontiguousarray(hwt_full[:, k * VSLICE:(k + 1) * VSLICE]),
        })
    return in_maps


def kernel(**inputs):
    global _CACHED_NC
    in_maps = prepare_in_maps(**inputs)
    if _CACHED_NC is None:
        _CACHED_NC = build_graph()
    nc = _CACHED_NC

    res = run_bass_kernel_spmd(nc, in_maps, core_ids=list(range(N_CORES)))
    global _LAST_RES
    _LAST_RES = res
    if not WITH_HEAD:
        return np.stack([r["xout"] for r in res.results])
    logits = np.concatenate([r["logits"] for r in res.results], axis=1)  # [2048, 32000]
    return logits.reshape(B, T, V).astype(np.float32)
